# revision 17
# baseline (speedup 1.0000x reference)
"""Trainium2 Bass kernel: peaking-EQ biquad (IIR) + volume gain + clamp.

The IIR filter's poles have radius ~0.925 (impulse response decays below f32
noise in <256 samples), so it is computed exactly (to ~3e-8) as a 256-tap FIR
convolution with the truncated impulse response, with the 0.8 amplitude gain
folded into the taps. The convolution runs on the tensor engine as
banded-Toeplitz matmuls over 128-sample blocks:

    y[c*128 + i] = sum_m sum_s G_m[s, i] * x[(c-m)*128 + s]
    G_m[s, i] = g[i - s + 128*m]   (3 bands cover taps 0..255)

Data-parallel over 8 NeuronCores (8 clips each). Per core, time blocks are
brought to the partition axis with PE transposes (in fp32r), convolved with
3 accumulating fp32r matmuls per 512-block segment, clamped on the vector
engine, transposed back, and DMA'd out. DMA traffic is batched into ~1.8MB
transfers (28 subtiles) to amortize per-DMA descriptor-generation overhead.
"""

import sys

import numpy as np

if "/opt/trn_rl_repo" not in sys.path:
    sys.path.insert(0, "/opt/trn_rl_repo")

import concourse.bacc as bacc
import concourse.tile as tile
from concourse import mybir
from concourse.bass_utils import run_bass_kernel_spmd
from concourse.mybir import AluOpType

F32 = mybir.dt.float32
F32R = mybir.dt.float32r

N_CORES = 8
B_TOTAL = 64
B_CORE = B_TOTAL // N_CORES  # clips per core
T = 960000
BLK = 128
NBLK = T // BLK  # 7500 blocks of 128 samples per clip
SEG = 512  # blocks per segment (PSUM fp32 bank limit)
GSUB = 14  # subtiles per DMA group (14*64KB = 896KB per transfer)
W = 256  # FIR taps kept (tail < 3e-8 relative)
NB = (W - 1 + BLK - 1) // BLK + 1  # Toeplitz bands: 3 for W=256
HIST = NB - 1  # history block-columns carried between segments

# Fixed augmentation parameters (match the reference module)
CENTER_FREQ = 1000.0
EQ_GAIN_DB = 3.0
Q = 0.707
VOL_GAIN = 0.8

assert NB == 3


def _coeffs(sr):
    """torchaudio equalizer_biquad coefficients, normalized by a0 (fp32)."""
    w0 = 2.0 * np.pi * CENTER_FREQ / sr
    A = 10.0 ** (EQ_GAIN_DB / 40.0)
    alpha = np.sin(w0) / (2.0 * Q)
    b0 = 1.0 + alpha * A
    b1 = -2.0 * np.cos(w0)
    b2 = 1.0 - alpha * A
    a0 = 1.0 + alpha / A
    a1 = -2.0 * np.cos(w0)
    a2 = 1.0 - alpha / A
    return tuple(np.float32(c / a0) for c in (b0, b1, b2, a1, a2))


def _taps(sr):
    """Truncated impulse response of the biquad, scaled by VOL_GAIN."""
    b0, b1, b2, a1, a2 = (np.float64(c) for c in _coeffs(sr))
    h = np.zeros(W, dtype=np.float64)
    for n in range(W):
        acc = 0.0
        if n == 0:
            acc += b0
        if n == 1:
            acc += b1
        if n == 2:
            acc += b2
        if n >= 1:
            acc -= a1 * h[n - 1]
        if n >= 2:
            acc -= a2 * h[n - 2]
        h[n] = acc
    return (VOL_GAIN * h).astype(np.float32)


def _g_mats(g):
    """GT[m][s, i] = g[i - s + 128*m] (lhsT layout: contraction dim first)."""
    i = np.arange(BLK)[None, :]
    s = np.arange(BLK)[:, None]
    gt = np.zeros((NB, BLK, BLK), dtype=np.float32)
    for m in range(NB):
        tap = i - s + BLK * m
        valid = (tap >= 0) & (tap < W)
        gt[m][valid] = g[tap[valid]]
    return gt


def _build_program(n_clips=B_CORE, nblk=NBLK, opts=None):
    """Trace + compile the per-core Bass program (SPMD across 8 cores)."""
    opts = dict(opts or {})
    bufs = opts.get("bufs", {})
    evict_t = opts.get("evict_t", "scalar")  # engine for psT evict
    evict_o = opts.get("evict_o", "scalar")  # engine for psO evict
    stage = opts.get("stage", 99)
    dma_out_eng = opts.get("dma_out", "sync")
    repeat = opts.get("repeat", 1)
    interleave = opts.get("interleave", 1)
    nc = bacc.Bacc("TRN2", target_bir_lowering=False, debug=False)
    t_len = nblk * BLK

    nsubf = nblk // BLK  # full 128-block subtiles per clip
    remb = nblk % BLK  # leftover blocks (partial subtile)
    nseg = (nblk + SEG - 1) // SEG

    # DMA groups: chunks of GSUB full subtiles; last group carries remainder.
    groups = []  # (startJ, nfull, has_rem)
    j0 = 0
    while j0 < nsubf or (j0 == nsubf == 0 and remb):
        nfull = min(GSUB, nsubf - j0)
        last = j0 + nfull == nsubf
        groups.append((j0, nfull, last and remb > 0))
        j0 += nfull
        if last:
            break
    n_groups = len(groups)

    def sub_home(J):
        """Global subtile J (nsubf == remainder) -> (group idx, col offset)."""
        if J >= nsubf:  # remainder pseudo-subtile
            g = n_groups - 1
            return g, groups[g][1] * BLK
        g = min(J // GSUB, n_groups - 1)
        return g, (J - groups[g][0]) * BLK

    x_d = nc.declare_dram_parameter("x", [n_clips, t_len], F32R, isOutput=False)
    gt_d = nc.declare_dram_parameter("gt", [NB, BLK, BLK], F32R, isOutput=False)
    id_d = nc.declare_dram_parameter("ident", [BLK, BLK], F32R, isOutput=False)
    z_d = nc.declare_dram_parameter("zeros", [BLK, HIST], F32R, isOutput=False)
    y_d = nc.declare_dram_parameter("y", [n_clips, t_len], F32R, isOutput=True)

    gmax = max(nf + (1 if hr else 0) for _, nf, hr in groups)

    with tile.TileContext(nc) as tc:
        with (
            tc.tile_pool(name="consts", bufs=1) as cpool,
            tc.tile_pool(name="xr", bufs=bufs.get("xr", 3)) as xrpool,
            tc.tile_pool(name="xt", bufs=bufs.get("xt", 4)) as xtpool,
            tc.tile_pool(name="yt", bufs=bufs.get("yt", 3)) as ytpool,
            tc.tile_pool(name="yo", bufs=bufs.get("yo", 3)) as yopool,
            tc.tile_pool(name="psT", bufs=bufs.get("psT", 3), space="PSUM") as psTpool,
            tc.tile_pool(name="psY", bufs=bufs.get("psY", 3), space="PSUM") as psYpool,
            tc.tile_pool(name="psO", bufs=bufs.get("psO", 2), space="PSUM") as psOpool,
        ):
            gt_t = cpool.tile([BLK, NB, BLK], F32R, tag="gt")
            nc.sync.dma_start(gt_t[:], gt_d[:].rearrange("m s i -> s m i"))
            id_t = cpool.tile([BLK, BLK], F32R, tag="ident")
            nc.sync.dma_start(id_t[:], id_d[:])
            z_t = cpool.tile([BLK, HIST], F32R, tag="zeros")
            nc.sync.dma_start(z_t[:], z_d[:])

            def load_group(c, gi, rep=0):
                """One batched in-DMA per group (plus one for the remainder)."""
                startJ, nfull, has_rem = groups[gi]
                xr = xrpool.tile([BLK, gmax * BLK], F32R, tag="xr", name=f"xr_{rep}_{c}_{gi}")
                if nfull:
                    nc.sync.dma_start(
                        xr[:, 0 : nfull * BLK].rearrange(
                            "b (j s) -> b j s", j=nfull
                        ),
                        x_d[
                            c,
                            startJ * BLK * BLK : (startJ + nfull) * BLK * BLK,
                        ].rearrange("(j b s) -> b j s", j=nfull, b=BLK, s=BLK),
                    )
                if has_rem:
                    nc.sync.dma_start(
                        xr[0:remb, nfull * BLK : (nfull + 1) * BLK],
                        x_d[c, nsubf * BLK * BLK : t_len].rearrange(
                            "(b s) -> b s", b=remb, s=BLK
                        ),
                    )
                return xr

            for rep in range(repeat):
              xr_tiles_c = {c: {} for c in range(n_clips)}
              prev_xt_c = {c: None for c in range(n_clips)}
              order = [
                  (c0 + ci, seg)
                  for c0 in range(0, n_clips, interleave)
                  for seg in range(nseg)
                  for ci in range(min(interleave, n_clips - c0))
              ]
              for c, seg in order:
                xr_tiles = xr_tiles_c[c]
                if True:
                    b0 = seg * SEG  # first block of segment
                    sblk = min(SEG, nblk - b0)  # blocks in this segment
                    # subtiles of this segment: (global J, col-in-seg, nblocks)
                    subs = []
                    bb = b0
                    while bb < b0 + sblk:
                        J = bb // BLK
                        jn = min(BLK, nblk - bb) if J >= nsubf else BLK
                        if J >= nsubf:
                            jn = remb
                        subs.append((J, (bb - b0), jn))
                        bb += jn

                    # --- ensure source groups are loaded ---
                    for J, _, _ in subs:
                        gi = sub_home(J)[0]
                        if gi not in xr_tiles:
                            xr_tiles[gi] = load_group(c, gi, rep)

                    if stage < 1:
                        continue
                    # --- transpose in: [block, sample] -> [sample, block] ---
                    psT = psTpool.tile([BLK, SEG], F32R, tag="psT")
                    for J, cs, jn in subs:
                        gi, col = sub_home(J)
                        nc.tensor.transpose(
                            psT[:, cs : cs + jn],
                            xr_tiles[gi][0:jn, col : col + BLK],
                            id_t[0:jn, 0:jn],
                        )

                    if stage < 2:
                        continue
                    xt = xtpool.tile([BLK, HIST + SEG], F32R, tag="xt")
                    if seg == 0:
                        nc.vector.tensor_copy(xt[:, 0:HIST], z_t[:])
                    else:
                        nc.vector.tensor_copy(
                            xt[:, 0:HIST], prev_xt_c[c][:, SEG : SEG + HIST]
                        )
                    _ev_t = (
                        nc.scalar.copy
                        if evict_t == "scalar"
                        else nc.vector.tensor_copy
                    )
                    _ev_t(xt[:, HIST : HIST + sblk], psT[:, 0:sblk])
                    prev_xt_c[c] = xt

                    if stage < 3:
                        continue
                    # --- banded conv: 3 accumulating fp32r matmuls ---
                    psY = psYpool.tile([BLK, SEG], F32, tag="psY")
                    for m in range(NB):
                        nc.tensor.matmul(
                            psY[:, 0:sblk],
                            gt_t[:, m, :],
                            xt[:, HIST - m : HIST - m + sblk],
                            start=(m == 0),
                            stop=(m == NB - 1),
                        )

                    if stage < 4:
                        continue
                    # --- clamp to [-1, 1] while evicting PSUM ---
                    yt = ytpool.tile([BLK, SEG], F32R, tag="yt")
                    nc.vector.tensor_scalar(
                        yt[:, 0:sblk],
                        psY[:, 0:sblk],
                        -1.0,
                        1.0,
                        AluOpType.max,
                        AluOpType.min,
                    )

                    if stage < 5:
                        continue
                    # --- transpose out: [sample, block] -> [block, sample] ---
                    psO = psOpool.tile([BLK, SEG], F32R, tag="psO")
                    for J, cs, jn in subs:
                        nc.tensor.transpose(
                            psO[0:jn, cs : cs + BLK],
                            yt[:, cs : cs + jn],
                            id_t[:],
                        )

                    if stage < 6:
                        continue
                    # --- evict transposed output + store (per segment) ---
                    if evict_o == "alt":
                        _ev_o = (
                            nc.scalar.copy if seg % 2 else nc.vector.tensor_copy
                        )
                    elif evict_o == "scalar":
                        _ev_o = nc.scalar.copy
                    else:
                        _ev_o = nc.vector.tensor_copy
                    yo = yopool.tile([BLK, SEG], F32R, tag="yo")
                    nfull_cols = sum(jn for _, _, jn in subs if jn == BLK)
                    if nfull_cols:
                        _ev_o(yo[:, 0:nfull_cols], psO[:, 0:nfull_cols])
                    has_part = subs[-1][2] != BLK
                    if has_part:
                        J, cs, jn = subs[-1]
                        _ev_o(yo[0:jn, cs : cs + BLK], psO[0:jn, cs : cs + BLK])

                    if stage < 7:
                        continue
                    _dma_o = nc.sync if dma_out_eng == "sync" else nc.scalar
                    if nfull_cols:
                        nj = nfull_cols // BLK
                        _dma_o.dma_start(
                            y_d[
                                c,
                                b0 * BLK : (b0 + nj * BLK) * BLK,
                            ].rearrange("(j b s) -> b j s", j=nj, b=BLK, s=BLK),
                            yo[:, 0:nfull_cols].rearrange(
                                "b (j s) -> b j s", j=nj
                            ),
                        )
                    if has_part:
                        J, cs, jn = subs[-1]
                        _dma_o.dma_start(
                            y_d[c, (b0 + cs) * BLK : (b0 + cs + jn) * BLK].rearrange(
                                "(b s) -> b s", b=jn, s=BLK
                            ),
                            yo[0:jn, cs : cs + BLK],
                        )

    nc.compile()
    return nc


_PROGRAM_CACHE = {}


def _get_program(n_clips=B_CORE, nblk=NBLK, opts=None):
    key = (n_clips, nblk, str(opts))
    if key not in _PROGRAM_CACHE:
        _PROGRAM_CACHE[key] = _build_program(n_clips, nblk, opts)
    return _PROGRAM_CACHE[key]


def kernel(waveform, sr, _trace=False):
    waveform = np.ascontiguousarray(np.asarray(waveform, dtype=np.float32))
    assert waveform.shape == (B_TOTAL, T), waveform.shape

    g = _taps(int(sr))
    gt = _g_mats(g)
    ident = np.eye(BLK, dtype=np.float32)

    nc = _get_program()
    in_maps = [
        {
            "x": waveform[c * B_CORE : (c + 1) * B_CORE],
            "gt": gt,
            "ident": ident,
            "zeros": np.zeros((BLK, HIST), dtype=np.float32),
        }
        for c in range(N_CORES)
    ]
    res = run_bass_kernel_spmd(nc, in_maps, list(range(N_CORES)), trace=_trace)
    out = np.concatenate([r["y"] for r in res.results], axis=0)
    if _trace:
        return out, res
    return out


# revision 19
# speedup vs baseline: 1.8051x; 1.8051x over previous
"""Trainium2 Bass kernel: peaking-EQ biquad (IIR) + volume gain + clamp.

The IIR filter's poles have radius ~0.925 (impulse response decays below f32
noise in <256 samples), so it is computed exactly (to ~3e-8) as a 256-tap FIR
convolution with the truncated impulse response, with the 0.8 amplitude gain
folded into the taps. The convolution runs on the tensor engine as
banded-Toeplitz matmuls over 128-sample blocks:

    y[c*128 + i] = sum_m sum_s G_m[s, i] * x[(c-m)*128 + s]
    G_m[s, i] = g[i - s + 128*m]   (3 bands cover taps 0..255)

Data-parallel over 8 NeuronCores (8 clips each). Per core, time blocks are
brought to the partition axis with PE transposes (in fp32r), convolved with
3 accumulating fp32r matmuls per 512-block segment, clamped on the vector
engine, transposed back, and DMA'd out. DMA traffic is batched into ~1.8MB
transfers (28 subtiles) to amortize per-DMA descriptor-generation overhead.
"""

import sys

import numpy as np

if "/opt/trn_rl_repo" not in sys.path:
    sys.path.insert(0, "/opt/trn_rl_repo")

import concourse.bacc as bacc
import concourse.tile as tile
from concourse import mybir
from concourse.bass_utils import run_bass_kernel_spmd
from concourse.mybir import AluOpType

F32 = mybir.dt.float32
F32R = mybir.dt.float32r

N_CORES = 8
B_TOTAL = 64
B_CORE = B_TOTAL // N_CORES  # clips per core
T = 960000
BLK = 128
NBLK = T // BLK  # 7500 blocks of 128 samples per clip
SEG = 512  # blocks per segment (PSUM fp32 bank limit)
GSUB = 12  # subtiles per DMA group (12*64KB = 768KB per transfer)
NBLK_PAD = 7680  # clips zero-padded to a multiple of SEG: uniform segments
W = 256  # FIR taps kept (tail < 3e-8 relative)
NB = (W - 1 + BLK - 1) // BLK + 1  # Toeplitz bands: 3 for W=256
HIST = NB - 1  # history block-columns carried between segments

# Fixed augmentation parameters (match the reference module)
CENTER_FREQ = 1000.0
EQ_GAIN_DB = 3.0
Q = 0.707
VOL_GAIN = 0.8

assert NB == 3


def _coeffs(sr):
    """torchaudio equalizer_biquad coefficients, normalized by a0 (fp32)."""
    w0 = 2.0 * np.pi * CENTER_FREQ / sr
    A = 10.0 ** (EQ_GAIN_DB / 40.0)
    alpha = np.sin(w0) / (2.0 * Q)
    b0 = 1.0 + alpha * A
    b1 = -2.0 * np.cos(w0)
    b2 = 1.0 - alpha * A
    a0 = 1.0 + alpha / A
    a1 = -2.0 * np.cos(w0)
    a2 = 1.0 - alpha / A
    return tuple(np.float32(c / a0) for c in (b0, b1, b2, a1, a2))


def _taps(sr):
    """Truncated impulse response of the biquad, scaled by VOL_GAIN."""
    b0, b1, b2, a1, a2 = (np.float64(c) for c in _coeffs(sr))
    h = np.zeros(W, dtype=np.float64)
    for n in range(W):
        acc = 0.0
        if n == 0:
            acc += b0
        if n == 1:
            acc += b1
        if n == 2:
            acc += b2
        if n >= 1:
            acc -= a1 * h[n - 1]
        if n >= 2:
            acc -= a2 * h[n - 2]
        h[n] = acc
    return (VOL_GAIN * h).astype(np.float32)


def _g_mats(g):
    """GT[m][s, i] = g[i - s + 128*m] (lhsT layout: contraction dim first)."""
    i = np.arange(BLK)[None, :]
    s = np.arange(BLK)[:, None]
    gt = np.zeros((NB, BLK, BLK), dtype=np.float32)
    for m in range(NB):
        tap = i - s + BLK * m
        valid = (tap >= 0) & (tap < W)
        gt[m][valid] = g[tap[valid]]
    return gt


def _build_program(n_clips=B_CORE, nblk=NBLK, opts=None):
    """Trace + compile the per-core Bass program (SPMD across 8 cores)."""
    opts = dict(opts or {})
    bufs = opts.get("bufs", {})
    evict_t = opts.get("evict_t", "scalar")  # engine for psT evict
    evict_o = opts.get("evict_o", "scalar")  # engine for psO evict
    stage = opts.get("stage", 99)
    dma_out_eng = opts.get("dma_out", "sync")
    repeat = opts.get("repeat", 1)
    interleave = opts.get("interleave", 1)
    hist_src = opts.get("hist_src", "xt")  # "xt" | "psT"
    nc = bacc.Bacc("TRN2", target_bir_lowering=False, debug=False)
    t_len = nblk * BLK

    nsubf = nblk // BLK  # full 128-block subtiles per clip
    remb = nblk % BLK  # leftover blocks (partial subtile)
    nseg = (nblk + SEG - 1) // SEG

    # DMA groups: chunks of GSUB full subtiles; last group carries remainder.
    groups = []  # (startJ, nfull, has_rem)
    j0 = 0
    while j0 < nsubf or (j0 == nsubf == 0 and remb):
        nfull = min(GSUB, nsubf - j0)
        last = j0 + nfull == nsubf
        groups.append((j0, nfull, last and remb > 0))
        j0 += nfull
        if last:
            break
    n_groups = len(groups)

    def sub_home(J):
        """Global subtile J (nsubf == remainder) -> (group idx, col offset)."""
        if J >= nsubf:  # remainder pseudo-subtile
            g = n_groups - 1
            return g, groups[g][1] * BLK
        g = min(J // GSUB, n_groups - 1)
        return g, (J - groups[g][0]) * BLK

    x_d = nc.declare_dram_parameter("x", [n_clips, t_len], F32R, isOutput=False)
    gt_d = nc.declare_dram_parameter("gt", [NB, BLK, BLK], F32R, isOutput=False)
    id_d = nc.declare_dram_parameter("ident", [BLK, BLK], F32R, isOutput=False)
    z_d = nc.declare_dram_parameter("zeros", [BLK, HIST], F32R, isOutput=False)
    y_d = nc.declare_dram_parameter("y", [n_clips, t_len], F32R, isOutput=True)

    gmax = max(nf + (1 if hr else 0) for _, nf, hr in groups)

    with tile.TileContext(nc) as tc:
        with (
            tc.tile_pool(name="consts", bufs=1) as cpool,
            tc.tile_pool(name="xr", bufs=bufs.get("xr", 3)) as xrpool,
            tc.tile_pool(name="xt", bufs=bufs.get("xt", 4)) as xtpool,
            tc.tile_pool(name="yt", bufs=bufs.get("yt", 3)) as ytpool,
            tc.tile_pool(name="yo", bufs=bufs.get("yo", 3)) as yopool,
            tc.tile_pool(name="psT", bufs=bufs.get("psT", 3), space="PSUM") as psTpool,
            tc.tile_pool(name="psY", bufs=bufs.get("psY", 3), space="PSUM") as psYpool,
            tc.tile_pool(name="psO", bufs=bufs.get("psO", 2), space="PSUM") as psOpool,
        ):
            gt_t = cpool.tile([BLK, NB, BLK], F32R, tag="gt")
            nc.sync.dma_start(gt_t[:], gt_d[:].rearrange("m s i -> s m i"))
            id_t = cpool.tile([BLK, BLK], F32R, tag="ident")
            nc.sync.dma_start(id_t[:], id_d[:])
            z_t = cpool.tile([BLK, HIST], F32R, tag="zeros")
            nc.sync.dma_start(z_t[:], z_d[:])

            def load_group(c, gi, rep=0):
                """One batched in-DMA per group (plus one for the remainder)."""
                startJ, nfull, has_rem = groups[gi]
                xr = xrpool.tile([BLK, gmax * BLK], F32R, tag="xr", name=f"xr_{rep}_{c}_{gi}")
                if nfull:
                    nc.sync.dma_start(
                        xr[:, 0 : nfull * BLK].rearrange(
                            "b (j s) -> b j s", j=nfull
                        ),
                        x_d[
                            c,
                            startJ * BLK * BLK : (startJ + nfull) * BLK * BLK,
                        ].rearrange("(j b s) -> b j s", j=nfull, b=BLK, s=BLK),
                    )
                if has_rem:
                    nc.sync.dma_start(
                        xr[0:remb, nfull * BLK : (nfull + 1) * BLK],
                        x_d[c, nsubf * BLK * BLK : t_len].rearrange(
                            "(b s) -> b s", b=remb, s=BLK
                        ),
                    )
                return xr

            for rep in range(repeat):
              xr_tiles_c = {c: {} for c in range(n_clips)}
              prev_xt_c = {c: None for c in range(n_clips)}
              prev_psT_c = {c: None for c in range(n_clips)}
              order = [
                  (c0 + ci, seg)
                  for c0 in range(0, n_clips, interleave)
                  for seg in range(nseg)
                  for ci in range(min(interleave, n_clips - c0))
              ]
              for c, seg in order:
                xr_tiles = xr_tiles_c[c]
                if True:
                    b0 = seg * SEG  # first block of segment
                    sblk = min(SEG, nblk - b0)  # blocks in this segment
                    # subtiles of this segment: (global J, col-in-seg, nblocks)
                    subs = []
                    bb = b0
                    while bb < b0 + sblk:
                        J = bb // BLK
                        jn = min(BLK, nblk - bb) if J >= nsubf else BLK
                        if J >= nsubf:
                            jn = remb
                        subs.append((J, (bb - b0), jn))
                        bb += jn

                    # --- ensure source groups are loaded ---
                    for J, _, _ in subs:
                        gi = sub_home(J)[0]
                        if gi not in xr_tiles:
                            xr_tiles[gi] = load_group(c, gi, rep)

                    if stage < 1:
                        continue
                    # --- transpose in: [block, sample] -> [sample, block] ---
                    psT = psTpool.tile([BLK, SEG], F32R, tag="psT")
                    for J, cs, jn in subs:
                        gi, col = sub_home(J)
                        nc.tensor.transpose(
                            psT[:, cs : cs + jn],
                            xr_tiles[gi][0:jn, col : col + BLK],
                            id_t[0:jn, 0:jn],
                        )

                    if stage < 2:
                        continue
                    xt = xtpool.tile([BLK, HIST + SEG], F32R, tag="xt")
                    if seg == 0:
                        nc.vector.tensor_copy(xt[:, 0:HIST], z_t[:])
                    elif hist_src == "psT":
                        nc.vector.tensor_copy(
                            xt[:, 0:HIST], prev_psT_c[c][:, SEG - HIST : SEG]
                        )
                    else:
                        nc.vector.tensor_copy(
                            xt[:, 0:HIST], prev_xt_c[c][:, SEG : SEG + HIST]
                        )
                    _ev_t = (
                        nc.scalar.copy
                        if evict_t == "scalar"
                        else nc.vector.tensor_copy
                    )
                    _ev_t(xt[:, HIST : HIST + sblk], psT[:, 0:sblk])
                    prev_xt_c[c] = xt
                    prev_psT_c[c] = psT

                    if stage < 3:
                        continue
                    # --- banded conv: 3 accumulating fp32r matmuls ---
                    psY = psYpool.tile([BLK, SEG], F32, tag="psY")
                    for m in range(NB):
                        nc.tensor.matmul(
                            psY[:, 0:sblk],
                            gt_t[:, m, :],
                            xt[:, HIST - m : HIST - m + sblk],
                            start=(m == 0),
                            stop=(m == NB - 1),
                        )

                    if stage < 4:
                        continue
                    # --- clamp to [-1, 1] while evicting PSUM ---
                    yt = ytpool.tile([BLK, SEG], F32R, tag="yt")
                    nc.vector.tensor_scalar(
                        yt[:, 0:sblk],
                        psY[:, 0:sblk],
                        -1.0,
                        1.0,
                        AluOpType.max,
                        AluOpType.min,
                    )

                    if stage < 5:
                        continue
                    # --- transpose out: [sample, block] -> [block, sample] ---
                    psO = psOpool.tile([BLK, SEG], F32R, tag="psO")
                    for J, cs, jn in subs:
                        nc.tensor.transpose(
                            psO[0:jn, cs : cs + BLK],
                            yt[:, cs : cs + jn],
                            id_t[:],
                        )

                    if stage < 6:
                        continue
                    # --- evict transposed output + store (per segment) ---
                    if evict_o == "alt":
                        _ev_o = (
                            nc.scalar.copy if seg % 2 else nc.vector.tensor_copy
                        )
                    elif evict_o == "scalar":
                        _ev_o = nc.scalar.copy
                    else:
                        _ev_o = nc.vector.tensor_copy
                    yo = yopool.tile([BLK, SEG], F32R, tag="yo")
                    nfull_cols = sum(jn for _, _, jn in subs if jn == BLK)
                    if nfull_cols:
                        _ev_o(yo[:, 0:nfull_cols], psO[:, 0:nfull_cols])
                    has_part = subs[-1][2] != BLK
                    if has_part:
                        J, cs, jn = subs[-1]
                        _ev_o(yo[0:jn, cs : cs + BLK], psO[0:jn, cs : cs + BLK])

                    if stage < 7:
                        continue
                    _dma_o = nc.sync if dma_out_eng == "sync" else nc.scalar
                    if nfull_cols:
                        nj = nfull_cols // BLK
                        _dma_o.dma_start(
                            y_d[
                                c,
                                b0 * BLK : (b0 + nj * BLK) * BLK,
                            ].rearrange("(j b s) -> b j s", j=nj, b=BLK, s=BLK),
                            yo[:, 0:nfull_cols].rearrange(
                                "b (j s) -> b j s", j=nj
                            ),
                        )
                    if has_part:
                        J, cs, jn = subs[-1]
                        _dma_o.dma_start(
                            y_d[c, (b0 + cs) * BLK : (b0 + cs + jn) * BLK].rearrange(
                                "(b s) -> b s", b=jn, s=BLK
                            ),
                            yo[0:jn, cs : cs + BLK],
                        )

    nc.compile()
    return nc


_PROGRAM_CACHE = {}


def _get_program(n_clips=B_CORE, nblk=NBLK_PAD, opts=None):
    key = (n_clips, nblk, str(opts))
    if key not in _PROGRAM_CACHE:
        _PROGRAM_CACHE[key] = _build_program(n_clips, nblk, opts)
    return _PROGRAM_CACHE[key]


def kernel(waveform, sr, _trace=False):
    waveform = np.ascontiguousarray(np.asarray(waveform, dtype=np.float32))
    assert waveform.shape == (B_TOTAL, T), waveform.shape

    g = _taps(int(sr))
    gt = _g_mats(g)
    ident = np.eye(BLK, dtype=np.float32)

    nc = _get_program()
    t_pad = NBLK_PAD * BLK
    xpad = np.zeros((B_TOTAL, t_pad), dtype=np.float32)
    xpad[:, :T] = waveform
    in_maps = [
        {
            "x": xpad[c * B_CORE : (c + 1) * B_CORE],
            "gt": gt,
            "ident": ident,
            "zeros": np.zeros((BLK, HIST), dtype=np.float32),
        }
        for c in range(N_CORES)
    ]
    res = run_bass_kernel_spmd(nc, in_maps, list(range(N_CORES)), trace=_trace)
    out = np.concatenate([r["y"] for r in res.results], axis=0)[:, :T]
    out = np.ascontiguousarray(out)
    if _trace:
        return out, res
    return out
